# revision 19
# baseline (speedup 1.0000x reference)
"""Trainium2 8-core attention kernel for nn_Attention_8409545965959.

Reference computation (B=4, N=2048, C=1024, H=16 heads, Dh=64):
    qkv = x @ Wqkv; q,k,v per head
    att = softmax(where(mask>0, -1e7, q @ k^T / sqrt(Dh)))
    out = (att @ v) @ Wproj + bproj

Masked keys contribute exactly zero to the softmax (exp underflows to 0
in f32), so K/V are compacted host-side to the unmasked tokens of each
batch, padded to a multiple of 128 (padded positions re-masked on device
via the exp bias). This is an exact reformulation that shrinks the
attention k-dimension from 2048 to ~1152.

Sharding: tensor-parallel on heads (2 heads/core, column-parallel Wqkv),
then an AllToAll reshards the attention output from head-parallel to
sequence-parallel, and each core computes full output rows (row-parallel
proj over its 1024-row slice). Final gather is host-side concatenation.

On-device dataflow (per core, heads h0=2c, h1=2c+1):
  - activations kept transposed: qT/kT [128ch, n] from Wq/Wk-stationary
    matmuls vs host-transposed x^T; v in normal layout [n, 128ch].
  - S^T[k,q] per head via row-group-packed matmul pairs (K=Dh=64,
    tile_position (0,0)/(64,0)), both heads' scores in one PSUM tile
    [128, 1024].
  - softmax: exp via ScalarE activation (scale=1/sqrt(Dh), per-partition
    bias = -30000 on masked/padded k rows -> exact zeros), E^T in bf16.
  - O^T += v_h^T @ E^T col-group-packed (M=64 at (0,0)/(0,64)); D (softmax
    denominators) via ones-matmuls (M=1 at partitions 0/32), both
    accumulated over the k-chunks in PSUM; O^T copied to SBUF right after
    the last chunk so the accumulator bank frees immediately.
  - normalization: 1/D via reciprocal_approx_fast, broadcast to 128
    partitions with a K=2 bf16 matmul against a selector, O^T * (1/D) on
    VectorE -> bf16.
  - Four per-batch AllToAlls reshard O^T (shard = 256 q rows per dest);
    proj is Wproj-stationary producing out^T [1024, 1024] per core
    (+bias), with each batch group's proj running as filler two batches
    later so only the last group sits in the tail.

To keep the PE dense (HAM clock-gate stays at 2.4 GHz only under
sustained activity) the emission order interleaves the next batch's QKV
matmuls (and the first proj half) into the attention inner loop as
independent filler work. A small startup AllToAll absorbs inter-core
launch stagger off the critical path.

kernel(**inputs) accepts the full unsharded inputs and returns the full
[4, 2048, 1024] float32 output.
"""

import sys
import types

import numpy as np
import ml_dtypes

# If a caller enables BASS_TRACE without the axon NTFF profiling hook
# installed, concourse's trace path would fail importing
# antenv.axon_hooks. Provide a no-op fallback (never overrides a real
# module) so tracing degrades gracefully instead of crashing.
try:
    import antenv.axon_hooks  # noqa: F401
except ImportError:
    try:
        import antenv

        _ah = types.ModuleType("antenv.axon_hooks")
        _ah._hook = None
        _ah.set_axon_ntff_profile_hook = lambda h: setattr(_ah, "_hook", h)
        _ah.get_axon_ntff_profile_hook = lambda: _ah._hook
        sys.modules["antenv.axon_hooks"] = _ah
        antenv.axon_hooks = _ah
    except ImportError:
        pass

import concourse.bass as bass
import concourse.mybir as mybir
import concourse.tile as tile
from concourse import bacc
from concourse.bass_utils import run_bass_kernel_spmd

B = 4
N = 2048
C = 1024
H = 16
NCORES = 8
DH = C // H            # 64
HPC = H // NCORES      # 2 heads per core -> 128 channels/core
CPC = HPC * DH         # 128
ROWS = B * N           # 8192
QB = 512               # q block (one PSUM bank of f32)
KCH = 128              # k chunk (partitions)
NQB = N // QB          # 4
CC = C // 128          # 8 contraction chunks
SCALE = DH ** -0.5     # 0.125
MASK_BIAS = -30000.0

DT = mybir.dt.float32
BF = mybir.dt.bfloat16
NPBF = ml_dtypes.bfloat16

_CACHE: dict = {}
LAST_RESULTS = None


def _build(nkc):
    """nkc = number of 128-row k-chunks after compaction (e.g. 9)."""
    nk = nkc * KCH
    nc = bacc.Bacc("TRN2", target_bir_lowering=False, debug=False, num_devices=NCORES)

    xT = nc.dram_tensor("xT", [C, ROWS], BF, kind="ExternalInput")
    xTk = nc.dram_tensor("xTk", [C, B * nk], BF, kind="ExternalInput")
    wq = nc.dram_tensor("wq", [C, CPC], BF, kind="ExternalInput")
    wk = nc.dram_tensor("wk", [C, CPC], BF, kind="ExternalInput")
    wv = nc.dram_tensor("wv", [C, CPC], BF, kind="ExternalInput")
    wp = nc.dram_tensor("wp", [C, C], BF, kind="ExternalInput")
    bvec = nc.dram_tensor("bvec", [128, CC], DT, kind="ExternalInput")
    mb = nc.dram_tensor("mb", [128, B * nkc], DT, kind="ExternalInput")
    sel2 = nc.dram_tensor("sel2", [2, 128], BF, kind="ExternalInput")
    out_ext = nc.dram_tensor("out", [C, 2 * QB], DT, kind="ExternalOutput")

    # k blocks for the K^T qkv matmuls (moving dim <= 512)
    kblocks = []
    pos = 0
    while pos < nk:
        w = min(QB, nk - pos)
        kblocks.append((pos, w))
        pos += w

    with tile.TileContext(nc) as tc:
        with (
            tc.tile_pool(name="consts", bufs=1) as consts,
            tc.tile_pool(name="xpool", bufs=2) as xpool,
            tc.tile_pool(name="kpool", bufs=2) as kpool,
            tc.tile_pool(name="qkpool", bufs=2) as qkpool,
            tc.tile_pool(name="vpool", bufs=2) as vpool,
            tc.tile_pool(name="epool", bufs=6) as epool,
            tc.tile_pool(name="npool", bufs=2) as npool,
            tc.tile_pool(name="opool", bufs=2) as opool,
            tc.tile_pool(name="dram", bufs=1, space="DRAM") as dram,
            tc.tile_pool(name="s_ps", bufs=2, space="PSUM") as s_ps,
            tc.tile_pool(name="o_ps", bufs=1, space="PSUM") as o_ps,
            tc.tile_pool(name="d_ps", bufs=1, space="PSUM") as d_ps,
            tc.tile_pool(name="aux_ps", bufs=2, space="PSUM") as aux_ps,
        ):
            # ---- persistent constants / weights
            wq_sb = consts.tile([128, CC, CPC], BF)
            wk_sb = consts.tile([128, CC, CPC], BF)
            wv_sb = consts.tile([128, CC, CPC], BF)
            wp_sb = consts.tile([128, CC, C], BF)
            bias_sb = consts.tile([128, CC], DT)
            mb_sb = consts.tile([128, B * nkc], DT)
            sel2_sb = consts.tile([2, 128], BF)
            ones_sb = consts.tile([128, 1], BF)
            nc.sync.dma_start(wq_sb[:], wq.rearrange("(cc p) m -> p cc m", p=128))
            nc.sync.dma_start(wk_sb[:], wk.rearrange("(cc p) m -> p cc m", p=128))
            nc.sync.dma_start(wv_sb[:], wv.rearrange("(cc p) m -> p cc m", p=128))
            nc.sync.dma_start(bias_sb[:], bvec[:])
            nc.sync.dma_start(mb_sb[:], mb[:])
            nc.sync.dma_start(sel2_sb[:], sel2[:])
            nc.vector.memset(ones_sb[:], 1.0)
            nc.sync.dma_start(wp_sb[:], wp.rearrange("(cc p) m -> p cc m", p=128))

            # AllToAll bounce buffers: one group per batch, shard = 256 q rows
            QS = QB // 2
            a2a_in = [
                dram.tile([NCORES, 128, QS], BF, name=f"a2a_in{i}", tag=f"a2a_in{i}")
                for i in range(B)
            ]
            a2a_out = [
                dram.tile([NCORES, 128, QS], BF, name=f"a2a_out{i}", tag=f"a2a_out{i}")
                for i in range(B)
            ]

            # startup alignment: absorb inter-core launch stagger on the
            # collective engine before real barriers sit on the critical path
            align_in = dram.tile([2, 4], BF, name="align_in", tag="align_in")
            align_out = dram.tile([2, 4], BF, name="align_out", tag="align_out")
            nc.sync.dma_start(align_in[:], sel2[0:2, 0:4])
            nc.gpsimd.collective_compute(
                "AllToAll",
                mybir.AluOpType.bypass,
                ins=[align_in.opt()],
                outs=[align_out.opt()],
                replica_groups=[list(range(NCORES))],
            )

            def emit_collective(grp):
                def emit():
                    nc.gpsimd.collective_compute(
                        "AllToAll",
                        mybir.AluOpType.bypass,
                        ins=[a2a_in[grp].opt()],
                        outs=[a2a_out[grp].opt()],
                        replica_groups=[list(range(NCORES))],
                    )

                return emit

            xb_tiles = {}
            kb_tiles = {}
            qkv_state = {}

            def emit_xb_load(b):
                xb = xpool.tile([128, CC, N], BF, name=f"xb{b}", tag="xb")
                xs = xT[:, b * N:(b + 1) * N].rearrange("(cc p) n -> p cc n", p=128)
                kb = kpool.tile([128, CC, nk], BF, name=f"kb{b}", tag="kb")
                ks = xTk[:, b * nk:(b + 1) * nk].rearrange("(cc p) n -> p cc n", p=128)
                for cc in range(CC):
                    nc.sync.dma_start(xb[:, cc, :], xs[:, cc, :])
                    nc.sync.dma_start(kb[:, cc, :], ks[:, cc, :])
                xb_tiles[b] = xb
                kb_tiles[b] = kb

            def qkv_units(b):
                """Independent emission units for batch b's QKV (filler work)."""
                xb = xb_tiles[b]
                kb = kb_tiles[b]
                qT = qkpool.tile([128, N], BF, name=f"qT{b}", tag="qT")
                kT = qkpool.tile([128, nk], BF, name=f"kT{b}", tag="kT")
                vt = vpool.tile([128, nkc, CPC], BF, name=f"vt{b}", tag="vt")
                qkv_state[b] = (qT, kT, vt)
                units = []

                def q_unit(rb):
                    def emit():
                        ps = aux_ps.tile([128, QB], DT, name=f"psq{b}_{rb}", tag="aux")
                        for cc in range(CC):
                            nc.tensor.matmul(
                                ps[:],
                                wq_sb[:, cc, :],
                                xb[:, cc, rb * QB:(rb + 1) * QB],
                                start=cc == 0,
                                stop=cc == CC - 1,
                            )
                        nc.vector.tensor_copy(qT[:, rb * QB:(rb + 1) * QB], ps[:])

                    return emit

                def k_unit(pos, w):
                    def emit():
                        ps = aux_ps.tile([128, QB], DT, name=f"psk{b}_{pos}", tag="aux")
                        for cc in range(CC):
                            nc.tensor.matmul(
                                ps[:, 0:w],
                                wk_sb[:, cc, :],
                                kb[:, cc, pos:pos + w],
                                start=cc == 0,
                                stop=cc == CC - 1,
                            )
                        nc.vector.tensor_copy(kT[:, pos:pos + w], ps[:, 0:w])

                    return emit

                def v_unit(rc):
                    def emit():
                        ps = aux_ps.tile([128, QB], DT, name=f"psv{b}_{rc}", tag="aux")
                        for cc in range(CC):
                            nc.tensor.matmul(
                                ps[:, 0:CPC],
                                kb[:, cc, rc * KCH:(rc + 1) * KCH],
                                wv_sb[:, cc, :],
                                start=cc == 0,
                                stop=cc == CC - 1,
                            )
                        nc.vector.tensor_copy(vt[:, rc, :], ps[:, 0:CPC])

                    return emit

                for rb in range(NQB):
                    units.append(q_unit(rb))
                for pos, w in kblocks:
                    units.append(k_unit(pos, w))
                for rc in range(nkc):
                    units.append(v_unit(rc))
                return units

            def attention_steps(b, carried=None):
                """One closure per (qb, kc) plus the per-qb normalization."""
                qT, kT, vt = qkv_state[b]
                kc_lists = [[] for _ in range(NQB)]
                norm_pairs = []
                for qb in range(NQB):
                    steps = kc_lists[qb]
                    o_acc = o_ps.tile([128, QB], DT, name=f"o{b}_{qb}", tag="o")
                    d_acc = d_ps.tile([128, QB], DT, name=f"d{b}_{qb}", tag="d")

                    for kc in range(nkc):
                        def kc_step(qb=qb, kc=kc, o_acc=o_acc, d_acc=d_acc):
                            s2 = s_ps.tile(
                                [128, 2 * QB], DT, name=f"s{b}_{qb}_{kc}", tag="s"
                            )
                            nc.tensor.matmul(
                                s2[:, 0:QB],
                                kT[0:DH, kc * KCH:(kc + 1) * KCH],
                                qT[0:DH, qb * QB:(qb + 1) * QB],
                                start=True,
                                stop=True,
                                tile_position=(0, 0),
                            )
                            nc.tensor.matmul(
                                s2[:, QB:2 * QB],
                                kT[DH:2 * DH, kc * KCH:(kc + 1) * KCH],
                                qT[DH:2 * DH, qb * QB:(qb + 1) * QB],
                                start=True,
                                stop=True,
                                tile_position=(64, 0),
                            )
                            e2 = epool.tile(
                                [128, 2 * QB], BF, name=f"e{b}_{qb}_{kc}", tag="e"
                            )
                            mcol = b * nkc + kc
                            nc.scalar.activation(
                                e2[:],
                                s2[:],
                                mybir.ActivationFunctionType.Exp,
                                bias=mb_sb[:, mcol:mcol + 1],
                                scale=SCALE,
                            )
                            st = kc == 0
                            sp = kc == nkc - 1
                            nc.tensor.matmul(
                                o_acc[0:DH, :],
                                vt[:, kc, 0:DH],
                                e2[:, 0:QB],
                                start=st,
                                stop=sp,
                                tile_position=(0, 0),
                            )
                            nc.tensor.matmul(
                                o_acc[DH:2 * DH, :],
                                vt[:, kc, DH:2 * DH],
                                e2[:, QB:2 * QB],
                                start=st,
                                stop=sp,
                                tile_position=(0, 64),
                            )
                            nc.tensor.matmul(
                                d_acc[0:1, :],
                                ones_sb[:],
                                e2[:, 0:QB],
                                start=st,
                                stop=sp,
                                tile_position=(0, 0),
                            )
                            nc.tensor.matmul(
                                d_acc[32:33, :],
                                ones_sb[:],
                                e2[:, QB:2 * QB],
                                start=st,
                                stop=sp,
                                tile_position=(0, 32),
                            )

                        steps.append(kc_step)

                    state = {}

                    def norm_a(qb=qb, o_acc=o_acc, d_acc=d_acc, state=state):
                        # free the PSUM accumulators immediately
                        osb = opool.tile([128, QB], DT, name=f"osb{b}_{qb}", tag="osb")
                        nc.vector.tensor_copy(osb[:], o_acc[:])
                        dstage = npool.tile([33, QB], DT, name=f"dst{b}_{qb}", tag="dstage")
                        nc.vector.tensor_copy(dstage[:], d_acc[0:33, :])
                        state["osb"] = osb
                        state["dstage"] = dstage

                    def norm_b(qb=qb, state=state):
                        # deferred: the dd-DMA/reciprocal chain latency hides
                        # behind the next q-block's attention matmuls
                        osb = state["osb"]
                        dstage = state["dstage"]
                        dd = npool.tile([2, QB], DT, name=f"dd{b}_{qb}", tag="dd")
                        nc.sync.dma_start(dd[0:1, :], dstage[0:1, :])
                        nc.sync.dma_start(dd[1:2, :], dstage[32:33, :])
                        dr = npool.tile([2, QB], DT, name=f"dr{b}_{qb}", tag="dr")
                        nc.vector.reciprocal_approx_fast(dr[:], dd[:])
                        drbf = npool.tile([2, QB], BF, name=f"drbf{b}_{qb}", tag="drbf")
                        nc.vector.tensor_copy(drbf[:], dr[:])
                        drb_ps = aux_ps.tile([128, QB], DT, name=f"drp{b}_{qb}", tag="aux")
                        nc.tensor.matmul(
                            drb_ps[:], sel2_sb[:], drbf[:], start=True, stop=True
                        )
                        of = opool.tile([128, QB], BF, name=f"of{b}_{qb}", tag="of")
                        nc.vector.tensor_mul(of[:], osb[:], drb_ps[:])
                        QS = QB // 2
                        nc.sync.dma_start(
                            a2a_in[b][2 * qb, :, :], of[:, 0:QS]
                        )
                        nc.sync.dma_start(
                            a2a_in[b][2 * qb + 1, :, :], of[:, QS:QB]
                        )

                    norm_pairs.append((norm_a, norm_b))
                # weave: kc-steps of qb, then norm_a(qb); norm_b(qb) lands
                # after the first 2 kc-steps of qb+1. The last qb's norm_b is
                # returned so the caller can hide it in the NEXT batch.
                woven = []
                pending_b = carried
                for qb in range(NQB):
                    for i in range(nkc):
                        woven.append(kc_lists[qb][i])
                        if i == 1 and pending_b is not None:
                            woven.append(pending_b)
                            pending_b = None
                    na, nb = norm_pairs[qb]
                    woven.append(na)
                    pending_b = nb
                return woven, pending_b

            def proj_units(grp):
                QS = QB // 2
                units = []

                def load_unit():
                    ofull = qkpool.tile(
                        [128, CC, QS], BF, name=f"ofull{grp}", tag="ofull"
                    )
                    qkv_state[f"ofull{grp}"] = ofull
                    nc.sync.dma_start(
                        ofull[:], a2a_out[grp].rearrange("i p j -> p i j")
                    )

                units.append(load_unit)

                def oc_unit(oc):
                    def emit():
                        ofull = qkv_state[f"ofull{grp}"]
                        pps = aux_ps.tile([128, QB], DT, name=f"pp{grp}_{oc}", tag="aux")
                        for cc in range(CC):
                            nc.tensor.matmul(
                                pps[:, 0:QS],
                                wp_sb[:, cc, oc * 128:(oc + 1) * 128],
                                ofull[:, cc, :],
                                start=cc == 0,
                                stop=cc == CC - 1,
                            )
                        fo = npool.tile([128, QS], DT, name=f"fo{grp}_{oc}", tag="fo")
                        nc.vector.tensor_scalar_add(
                            fo[:], pps[:, 0:QS], bias_sb[:, oc:oc + 1]
                        )
                        nc.sync.dma_start(
                            out_ext[oc * 128:(oc + 1) * 128, grp * QS:(grp + 1) * QS],
                            fo[:],
                        )

                    return emit

                for oc in range(CC):
                    units.append(oc_unit(oc))
                return units

            def run_interleaved(steps, fillers, clump=1):
                # emit fillers in clumps so the PE sees contiguous multi-us
                # bursts (the HAM clock-gate needs a gap-free busy window to
                # lift the 1.2 GHz throttle)
                nf = len(fillers)
                ns = len(steps)
                fi = 0
                for i, s in enumerate(steps):
                    s()
                    if fi < nf and (i + 1) * nf >= (fi + 1) * ns:
                        for _ in range(clump):
                            if fi < nf:
                                fillers[fi]()
                                fi += 1
                while fi < nf:
                    fillers[fi]()
                    fi += 1

            # ---- schedule: collective for batch b is emitted early in batch
            # b+1's attention; proj for group g runs as filler in batch g+2
            # (by then the collective has certainly completed). Batch 0 starts
            # attention as soon as kT/v[0..4]/q0 exist; the rest of its QKV
            # runs as early filler.
            emit_xb_load(0)
            units0 = qkv_units(0)
            # units order: q0..q3, k-blocks, v0..v8
            nq = NQB
            nkb = len(kblocks)
            upfront = units0[nq:nq + nkb] + units0[nq + nkb:nq + nkb + 5] + [units0[0]]
            rest0 = units0[nq + nkb + 5:] + units0[1:nq]
            for u in upfront:
                u()
            pending = None
            carried = None
            for b in range(B):
                fillers = []
                if b == 0:
                    fillers.extend(rest0)
                if b < B - 1:
                    emit_xb_load(b + 1)
                    fillers.extend(qkv_units(b + 1))
                if b >= 2:
                    fillers.extend(proj_units(b - 2))
                steps, carried = attention_steps(b, carried)
                if pending is not None:
                    steps.insert(4, pending)
                    pending = None
                run_interleaved(steps, fillers)
                pending = emit_collective(b)
            if carried is not None:
                carried()
            pending()
            for g in (2, 3):
                for u in proj_units(g):
                    u()

    nc.compile()
    return nc


def _prep_inputs(x, Wqkv, Wproj, bproj, mask, nkc):
    x = np.asarray(x, dtype=np.float32)
    Wqkv = np.asarray(Wqkv, dtype=np.float32)
    Wproj = np.asarray(Wproj, dtype=np.float32)
    bproj = np.asarray(bproj, dtype=np.float32)
    mask = np.asarray(mask)
    nk = nkc * KCH

    x2 = x.reshape(ROWS, C)
    xT = np.ascontiguousarray(x2.T).astype(NPBF)
    # compacted K/V tokens: unmasked columns per batch, zero-padded to nk
    xTk = np.zeros((C, B * nk), dtype=NPBF)
    mbias = np.full((B, nk), np.float32(MASK_BIAS), dtype=np.float32)
    for b in range(B):
        idx = np.nonzero(mask[b] == 0)[0]
        cnt = len(idx)
        xTk[:, b * nk: b * nk + cnt] = xT[:, b * N + idx]
        mbias[b, :cnt] = 0.0
    mb_arr = np.ascontiguousarray(
        mbias.reshape(B, nkc, 128).transpose(2, 0, 1).reshape(128, B * nkc)
    ).astype(np.float32)

    wp_bf = Wproj.astype(NPBF)
    bias_r = np.ascontiguousarray(bproj.reshape(CC, 128).T).astype(np.float32)
    sel2 = np.zeros((2, 128), np.float32)
    sel2[0, 0:64] = 1.0
    sel2[1, 64:128] = 1.0
    sel2 = sel2.astype(NPBF)

    in_maps = []
    for c in range(NCORES):
        cols = slice(c * CPC, (c + 1) * CPC)
        in_maps.append(
            dict(
                xT=xT,
                xTk=xTk,
                wq=np.ascontiguousarray(Wqkv[:, cols]).astype(NPBF),
                wk=np.ascontiguousarray(Wqkv[:, C:][:, cols]).astype(NPBF),
                wv=np.ascontiguousarray(Wqkv[:, 2 * C:][:, cols]).astype(NPBF),
                wp=wp_bf,
                bvec=bias_r,
                mb=mb_arr,
                sel2=sel2,
            )
        )
    return in_maps


def kernel(x, Wqkv, Wproj, bproj, mask):
    global LAST_RESULTS
    mask = np.asarray(mask)
    max_unmasked = int((mask == 0).sum(axis=1).max())
    nkc = max(1, -(-max_unmasked // KCH))
    if nkc not in _CACHE:
        _CACHE[nkc] = _build(nkc)
    nc = _CACHE[nkc]
    in_maps = _prep_inputs(x, Wqkv, Wproj, bproj, mask, nkc)
    res = run_bass_kernel_spmd(nc, in_maps, list(range(NCORES)))
    LAST_RESULTS = res
    out = np.empty((ROWS, C), dtype=np.float32)
    QS = QB // 2
    for c in range(NCORES):
        oT = res.results[c]["out"]  # [1024 oc, 4*256 q] = final^T slice
        for b in range(B):
            rows = slice(b * N + c * QS, b * N + (c + 1) * QS)
            out[rows, :] = oT[:, b * QS:(b + 1) * QS].T
    return out.reshape(B, N, C)
